# revision 34
# baseline (speedup 1.0000x reference)
import sys

sys.path.insert(0, "/opt/trn_rl_repo")

import numpy as np
import ml_dtypes
import concourse.bass as bass
import concourse.mybir as mybir
from concourse.bass_utils import run_bass_kernel_spmd
from concourse.tile import TileContext

FP32 = mybir.dt.float32
F32R = mybir.dt.float32r
BF16 = mybir.dt.bfloat16
BFNP = ml_dtypes.bfloat16

B, T, C = 2, 2048, 1024
H, DK = 16, 64
NCORES = 8
HPC = 4
TB = T // 128
CB = C // 128
NCH = T // 512
GROUPS = [[0, 1, 2, 3], [4, 5, 6, 7]]
NEG = -8.0e9

_CACHE = {}


def _split_excess_waits(nc):
    ctr = 0
    for f in nc.m.functions:
        for bb in f.blocks:
            new_insts = []
            changed = False
            for inst in bb.instructions:
                si = inst.sync_info
                if si is not None and si.on_wait and len(si.on_wait) > 1:
                    waits = list(si.on_wait)
                    for w in waits[:-1]:
                        ctr += 1
                        nop = mybir.InstNoOp(
                            name=f"I-waitsplit-{ctr}", ins=[], outs=[]
                        )
                        nop.engine = inst.engine
                        nop.sync_info = mybir.SyncInfo(on_wait=[w], on_update=[])
                        new_insts.append(nop)
                        changed = True
                    inst.sync_info = mybir.SyncInfo(
                        on_wait=[waits[-1]],
                        on_update=list(si.on_update) if si.on_update else [],
                    )
                new_insts.append(inst)
            if changed:
                bb.instructions = new_insts
    return ctr


def build_program(n_reps=1, loop_always=False, parts="ABC", dbg=False):
    nc = bass.Bass("TRN2", target_bir_lowering=False, debug=False,
                   num_devices=NCORES)

    xt = nc.declare_dram_parameter("xt", [C, T], BF16, isOutput=False)
    wq = nc.declare_dram_parameter("wq", [C, HPC * DK], BF16, isOutput=False)
    wk = nc.declare_dram_parameter("wk", [C, HPC * DK], BF16, isOutput=False)
    wv = nc.declare_dram_parameter("wv", [C, HPC * DK], BF16, isOutput=False)
    wot = nc.declare_dram_parameter("wot", [2 * 128, C], BF16, isOutput=False)
    bob = nc.declare_dram_parameter("bob", [128, CB], FP32, isOutput=False)
    y = nc.declare_dram_parameter("y", [C // 4, T], BF16, isOutput=True)

    mask_np = np.where(
        np.arange(128)[None, :] < np.arange(128)[:, None], NEG, 0.0
    ).astype(BFNP)
    eye_np = np.eye(128, dtype=BFNP)
    maskp = nc.inline_tensor(mask_np, name="maskp")
    eyep = nc.inline_tensor(eye_np, name="eyep")

    ypT = nc.dram_tensor("ypT", [C, T], BF16)
    ysT = nc.dram_tensor("ysT", [C // 4, T], BF16)
    DBG = {}
    if dbg:
        for nm, shp in (("d_qt0", [128, T]), ("d_kt0", [128, T]),
                        ("d_v50", [128, HPC * 65]), ("d_pt00", [128, T]),
                        ("d_pt03", [128, T]),
                        ("d_out0", [128, T]), ("d_out1", [128, T]),
                        ("d_yp", [C, T])):
            DBG[nm] = nc.declare_dram_parameter(nm, shp, BF16, isOutput=True)
    nc._DBG = DBG

    with TileContext(nc) as tc:
        with (
            tc.tile_pool(name="const", bufs=1) as pc,
            tc.tile_pool(name="wts", bufs=1) as pw,
            tc.tile_pool(name="xtp", bufs=1) as px,
            tc.tile_pool(name="qk", bufs=1) as pqk,
            tc.tile_pool(name="v5p", bufs=1) as pv5,
            tc.tile_pool(name="outp", bufs=1) as pout,
        ):
            maskt = pc.tile([128, 128], BF16, name="maskt")
            nc.sync.dma_start(out=maskt[:], in_=maskp[:])
            eyet = pc.tile([128, 128], BF16, name="eyet")
            nc.sync.dma_start(out=eyet[:], in_=eyep[:])
            onesb = pc.tile([65, DK], FP32, name="onesb")
            nc.vector.memset(onesb[:], 1.0)
            bot = pc.tile([128, CB], FP32, name="bot")
            nc.sync.dma_start(out=bot[:], in_=bob[:])

            QT = [pqk.tile([128, T], BF16, tag=f"qt{p}", name=f"qt{p}")
                  for p in range(2)]
            KT = [pqk.tile([128, T], BF16, tag=f"kt{p}", name=f"kt{p}")
                  for p in range(2)]
            V5 = [pv5.tile([128, HPC * 65], BF16, tag=f"v5_{tt}",
                           name=f"v5_{tt}") for tt in range(TB)]
            OUTT = [pout.tile([128, T], BF16, tag=f"out{p}", name=f"out{p}")
                    for p in range(2)]
            XTT = [px.tile([128, T], BF16, tag=f"xt{cb}", name=f"xt{cb}")
                   for cb in range(CB)]
            WQT = [pw.tile([128, HPC * DK], BF16, tag=f"wq{cb}",
                           name=f"wqt{cb}") for cb in range(CB)]
            WKT = [pw.tile([128, HPC * DK], BF16, tag=f"wk{cb}",
                           name=f"wkt{cb}") for cb in range(CB)]
            WVT = [pw.tile([128, HPC * DK], BF16, tag=f"wv{cb}",
                           name=f"wvt{cb}") for cb in range(CB)]
            WOT = [pw.tile([128, C], BF16, tag=f"wo{kb}", name=f"wot{kb}")
                   for kb in range(2)]

            def body():
                _emit_AB(nc, tc, xt, wq, wk, wv, maskt, eyet, onesb,
                         QT, KT, V5, OUTT, XTT, WQT, WKT, WVT, WOT, wot,
                         parts)
                if "C" in parts:
                    _emit_C(nc, tc, WOT, OUTT, bot, ypT)

            if n_reps == 1 and not loop_always:
                body()
            else:
                hint = (mybir.EngineType.PE, mybir.EngineType.Activation)
                with tc.For_i(0, n_reps, 1, hint_engines=hint) as _i:
                    body()
            _emit_ccout(nc, tc, ypT, ysT, y)

    _split_excess_waits(nc)
    return nc


def _emit_ccout(nc, tc, ypT, ysT, y):
    with tc.tile_pool(name="ccb", bufs=2) as pcc:
        nc.gpsimd.collective_compute(
            "ReduceScatter", mybir.AluOpType.add,
            ins=[ypT[:]], outs=[ysT[:]], replica_groups=GROUPS)
        for i in range(2):
            t = pcc.tile([128, T], BF16, tag="ybounce", name=f"yb{i}")
            nc.sync.dma_start(out=t[:], in_=ysT[i * 128:(i + 1) * 128, :])
            nc.sync.dma_start(out=y[i * 128:(i + 1) * 128, :], in_=t[:])


def _emit_AB(nc, tc, xt, wq, wk, wv, maskt, eyet, onesb,
             QT, KT, V5, OUTT, XTT, WQT, WKT, WVT, WOT, wot,
             parts):
    with (
        tc.tile_pool(name="psA", bufs=1, space="PSUM") as ppa,
    ):
        for cb in range(CB):
            eng = nc.sync
            eng.dma_start(out=WQT[cb][:], in_=wq[cb * 128:(cb + 1) * 128, :])
            eng.dma_start(out=WKT[cb][:], in_=wk[cb * 128:(cb + 1) * 128, :])
            eng.dma_start(out=XTT[cb][:], in_=xt[cb * 128:(cb + 1) * 128, :])
        for cb in range(CB):
            eng = nc.sync
            eng.dma_start(out=WVT[cb][:], in_=wv[cb * 128:(cb + 1) * 128, :])

        if "A" in parts:
            for W, DST, nm in ((WQT, QT, "q"), (WKT, KT, "k")):
                ps = {}
                for p in range(2):
                    for tch in range(NCH):
                        ps[(p, tch)] = ppa.tile(
                            [128, 512], FP32, tag=f"qk{p}{tch}", bufs=1,
                            name=f"ps{nm}{p}{tch}")
                for cb in range(CB):
                    for p in range(2):
                        for tch in range(NCH):
                            nc.tensor.matmul(
                                ps[(p, tch)][:],
                                W[cb][:, p * 128:(p + 1) * 128],
                                XTT[cb][:, tch * 512:(tch + 1) * 512],
                                start=(cb == 0), stop=(cb == CB - 1))
                for p in range(2):
                    for tch in range(NCH):
                        nc.scalar.copy(
                            DST[p][:, tch * 512:(tch + 1) * 512],
                            ps[(p, tch)][:])

            for tt in range(TB):
                psv = ppa.tile([128, HPC * DK], FP32, tag=f"qk0{tt % 2}",
                               bufs=1, name=f"psv{tt}")
                for cb in range(CB):
                    nc.tensor.matmul(
                        psv[:],
                        XTT[cb][:, tt * 128:(tt + 1) * 128],
                        WVT[cb][:],
                        start=(cb == 0), stop=(cb == CB - 1))
                for h in range(HPC):
                    nc.scalar.copy(
                        V5[tt][:, 65 * h:65 * h + 64],
                        psv[:, h * 64:(h + 1) * 64])
                    nc.vector.memset(V5[tt][:, 65 * h + 64:65 * h + 65], 1.0)


    with (
        tc.tile_pool(name="ptp", bufs=3) as ppt,
        tc.tile_pool(name="stgB", bufs=3) as pst,
        tc.tile_pool(name="psS", bufs=2, space="PSUM") as pps,
        tc.tile_pool(name="psO", bufs=1, space="PSUM") as ppo,
    ):
        for kb in range(2):
            nc.sync.dma_start(out=WOT[kb][:],
                                in_=wot[kb * 128:(kb + 1) * 128, :])

        if "B" in parts:
            PTs = {}
            ps_ocs = {}

            def emit_scores(h, jj):
                p, r0 = h // 2, (h % 2) * 64
                qh = QT[p][r0:r0 + 64, :]
                kh = KT[p][r0:r0 + 64, :]
                t0 = jj * 128
                PT = ppt.tile([128, T], BF16, tag="pt", name=f"pt{h}_{jj}")
                PTs[(h, jj)] = PT
                w0 = (t0 // 1024) * 1024
                diag_w0 = w0
                while w0 < T:
                    cs = max(t0, w0)
                    ce = w0 + 1024
                    ps_s = pps.tile([128, 1024], FP32, tag="ps",
                                    name=f"ps{h}_{jj}_{w0}")
                    col = cs
                    while col < ce:
                        sub = min(512 - (col % 512), ce - col)
                        last_sub = (col + sub >= ce)
                        nc.tensor.matmul(
                            ps_s[:, col - w0:col - w0 + sub],
                            kh[:, t0:t0 + 128],
                            qh[:, col:col + sub],
                            start=True,
                            stop=(last_sub and w0 != diag_w0))
                        col += sub
                    if w0 == diag_w0:
                        nc.tensor.matmul(
                            ps_s[:, cs - w0:cs - w0 + 128],
                            eyet[:],
                            maskt[:],
                            start=False, stop=True)
                    nc.scalar.activation(
                        PT[:, cs:ce], ps_s[:, cs - w0:ce - w0],
                        mybir.ActivationFunctionType.Exp, scale=0.125)
                    w0 += 1024

            def emit_attnv(h, jj):
                p, r0 = h // 2, (h % 2) * 64
                if jj == 0:
                    ps_ocs[h] = [
                        ppo.tile([128, 512], FP32, tag=f"oc{c}",
                                 name=f"oc{c}_{h}")
                        for c in range(NCH)]
                ps_oc = ps_ocs[h]
                t0 = jj * 128
                PT = PTs.pop((h, jj))
                for c in range(jj // 4, NCH):
                    lo = max(t0, c * 512)
                    nc.tensor.matmul(
                        ps_oc[c][0:65, lo - c * 512:512],
                        V5[jj][:, 65 * h:65 * h + 65],
                        PT[:, lo:(c + 1) * 512],
                        start=(jj == 0), stop=(jj == 4 * c + 3))
                    if jj == 4 * c + 3:
                        raw = pst.tile([64, 512], FP32, tag="raw",
                                       name=f"raw{h}_{c}")
                        nc.vector.tensor_copy(raw[:], ps_oc[c][0:64, :])
                        rect = pst.tile([65, 512], F32R, tag="rect",
                                        name=f"rect{h}_{c}")
                        with nc.allow_low_precision(reason="f32r recip"):
                            nc.vector.reciprocal(
                                rect[64:65, :], ps_oc[c][64:65, :])
                        nc.tensor.matmul(
                            ps_oc[c][0:64, :],
                            onesb[64:65, :].bitcast(F32R),
                            rect[64:65, :],
                            start=True, stop=True)
                        if h % 2 == 0:
                            odst = OUTT[p][0:64, c * 512:(c + 1) * 512]
                        else:
                            odst = pst.tile([64, 512], BF16, tag="ostg",
                                            name=f"ostg{h}_{c}")[:]
                        with nc.allow_low_precision(reason="bf16 out"):
                            nc.vector.tensor_tensor(
                                out=odst,
                                in0=raw[:], in1=ps_oc[c][0:64, :],
                                op=mybir.AluOpType.mult)
                        if h % 2 == 1:
                            eng = nc.sync
                            eng.dma_start(
                                out=OUTT[p][64:128, c * 512:(c + 1) * 512],
                                in_=odst)

            steps = [(h, jj) for h in range(HPC) for jj in range(TB)]
            for i in range(len(steps) + 1):
                if i < len(steps):
                    emit_scores(*steps[i])
                    DBG = getattr(nc, "_DBG", {})
                    if DBG and steps[i] == (0, 0):
                        nc.sync.dma_start(out=DBG["d_pt00"][:],
                                          in_=PTs[(0, 0)][:])
                    if DBG and steps[i] == (0, 3):
                        nc.sync.dma_start(out=DBG["d_pt03"][:],
                                          in_=PTs[(0, 3)][:])
                if i >= 1:
                    emit_attnv(*steps[i - 1])
            DBG = getattr(nc, "_DBG", {})
            if DBG:
                nc.sync.dma_start(out=DBG["d_qt0"][:], in_=QT[0][:])
                nc.sync.dma_start(out=DBG["d_kt0"][:], in_=KT[0][:])
                nc.sync.dma_start(out=DBG["d_v50"][:], in_=V5[0][:])
                nc.sync.dma_start(out=DBG["d_out0"][:], in_=OUTT[0][:])
                nc.sync.dma_start(out=DBG["d_out1"][:], in_=OUTT[1][:])


def _emit_C(nc, tc, WOT, OUTT, bot, ypT):
    with (
        tc.tile_pool(name="ysb", bufs=2) as pys,
        tc.tile_pool(name="psC", bufs=2, space="PSUM") as ppc,
    ):
        for cb in range(CB):
            ysw = pys.tile([128, T], BF16, tag="ysw", name=f"ysw{cb}")
            for tp in range(2):
                ps_y = ppc.tile([128, 1024], FP32, tag="psy",
                                name=f"psy{cb}_{tp}")
                for ti in range(2):
                    tch = tp * 2 + ti
                    for kb in range(2):
                        nc.tensor.matmul(
                            ps_y[:, ti * 512:(ti + 1) * 512],
                            WOT[kb][:, cb * 128:(cb + 1) * 128],
                            OUTT[kb][:, tch * 512:(tch + 1) * 512],
                            start=(kb == 0), stop=(kb == 1))
                dst = ysw[:, tp * 1024:(tp + 1) * 1024]
                if (cb + tp) % 2 == 0:
                    nc.scalar.activation(
                        dst, ps_y[:],
                        mybir.ActivationFunctionType.Identity,
                        bias=bot[:, cb:cb + 1])
                else:
                    nc.vector.tensor_scalar(
                        out=dst, in0=ps_y[:],
                        scalar1=bot[:, cb:cb + 1], scalar2=None,
                        op0=mybir.AluOpType.add)
            eng = nc.sync
            eng.dma_start(out=ypT[cb * 128:(cb + 1) * 128, :], in_=ysw[:])
            DBG = getattr(nc, "_DBG", {})
            if DBG:
                nc.sync.dma_start(
                    out=DBG["d_yp"][cb * 128:(cb + 1) * 128, :], in_=ysw[:])


def _make_in_maps(x, Wq, Wk, Wv, Wo, bo):
    in_maps = []
    for c in range(NCORES):
        b, hh = c // 4, HPC * (c % 4)
        ch0 = hh * DK
        in_maps.append({
            "xt": np.ascontiguousarray(x[b].T).astype(BFNP),
            "wq": np.concatenate([Wq[hh + i] for i in range(HPC)],
                                 axis=1).astype(BFNP),
            "wk": np.concatenate([Wk[hh + i] for i in range(HPC)],
                                 axis=1).astype(BFNP),
            "wv": np.concatenate([Wv[hh + i] for i in range(HPC)],
                                 axis=1).astype(BFNP),
            "wot": np.ascontiguousarray(
                Wo[:, ch0:ch0 + HPC * DK].T).astype(BFNP),
            "bob": np.ascontiguousarray(
                (bo / 4.0).reshape(CB, 128).T).astype(np.float32),
        })
    return in_maps


def kernel(x, Wq, Wk, Wv, Wo, bo):
    x = np.asarray(x, dtype=np.float32)
    Wq = np.asarray(Wq, dtype=np.float32)
    Wk = np.asarray(Wk, dtype=np.float32)
    Wv = np.asarray(Wv, dtype=np.float32)
    Wo = np.asarray(Wo, dtype=np.float32)
    bo = np.asarray(bo, dtype=np.float32)

    if "nc" not in _CACHE:
        _CACHE["nc"] = build_program()
    nc = _CACHE["nc"]

    in_maps = _make_in_maps(x, Wq, Wk, Wv, Wo, bo)
    res = run_bass_kernel_spmd(nc, in_maps, list(range(NCORES)))

    out = np.empty((B, T, C), dtype=np.float32)
    for c in range(NCORES):
        b, g = c // 4, c % 4
        ys = np.asarray(res.results[c]["y"])
        out[b, :, g * 256:(g + 1) * 256] = ys.T.astype(np.float32)
    return out
